# revision 8
# baseline (speedup 1.0000x reference)
"""CrossDomainAttention TRN2 kernel: 8-core data-parallel over batch.

Reference computation (per batch element, a/b are (L, C) slices):
  ap = a.T (C, L);  q = ap@Wq.T+bq; k,v from b.T
  attn = softmax(q @ k.T / sqrt(L)) (C, C)
  out = LN(attn @ v + ap) over L, returned as the raw (C*L) buffer viewed (L, C)

v2: fp8 (e4m3) DoubleRow with a 2-deep software pipeline keeping the PE
dense (HAM stays warm).  Each attention "round" interleaves on the PE:
scores(stage s, dp) + PE row-sum(dp-1) + one PV chain of stage s-1, so
the Act engine's exp drain (~2.2us/round) always fits inside the PE
round (~3.8us).  The next element's DMA/casts/transposes/projections are
emitted as fillers inside earlier stages (a-side into p=0, b-side+proj
into p=1).  LN uses the scale-invariant form (out_pre = rowsum*apT + PV)
with a batched Newton rsqrt.  Residual apT is fp16, transposed from an
fp16 copy of a at 1 cyc/row.
"""

import numpy as np

B, L, C = 16, 512, 2048
NCORE = 8
NB = B // NCORE          # batch elements per core
P = 128
F = 512                  # matmul free-dim tile
NLC = L // P             # 4  l/m chunks
NDB = C // P             # 16 d-blocks / c-blocks
NCCH = C // F            # 4  c chunks
NDP = NDB // 2           # 8  d-pairs (DoubleRow)
NLP = NLC // 2           # 2  l/m pairs (DoubleRow)
LN_EPS = 1e-5
RSTD_SEED = 4.77e-4   # ~1/sqrt(mean var') for the scale-invariant LN form
INV_SQRT_L = 1.0 / float(np.sqrt(L))

_CACHE = {}


def _build(apply_qkv_bias: bool, apply_gamma_beta: bool, repeat: int = 1):
    import concourse.bass as bass
    import concourse.tile as tile
    from concourse import bacc, mybir
    from concourse.bass import ts, ds
    from concourse.masks import make_identity
    from contextlib import ExitStack

    f32 = mybir.dt.float32
    f16 = mybir.dt.float16
    f8 = mybir.dt.float8e4
    AF = mybir.ActivationFunctionType
    ALU = mybir.AluOpType
    DR = mybir.MatmulPerfMode.DoubleRow

    nc = bacc.Bacc("TRN2", target_bir_lowering=False, debug=False,
                   enable_asserts=False)

    a_d = nc.dram_tensor("a", (NB, L, C), f32, kind="ExternalInput").ap()
    b_d = nc.dram_tensor("b", (NB, L, C), f32, kind="ExternalInput").ap()
    w_d = {n: nc.dram_tensor(n, (L, L), f32, kind="ExternalInput").ap()
           for n in ("Wq", "Wk", "Wv")}
    bias_d = {n: nc.dram_tensor(n, (L,), f32, kind="ExternalInput").ap()
              for n in ("bq", "bk", "bv")}
    gamma_d = nc.dram_tensor("gamma", (L,), f32, kind="ExternalInput").ap()
    beta_d = nc.dram_tensor("beta", (L,), f32, kind="ExternalInput").ap()
    out_d = nc.dram_tensor("out", (NB, C, L), f32, kind="ExternalOutput").ap()

    def bcast_p(ap1d):
        # broadcast a 1-D DRAM AP across all 128 partitions (DMA source)
        return bass.AP(tensor=ap1d.tensor, offset=ap1d.offset,
                       ap=[[0, P]] + [list(d) for d in ap1d.ap])

    ELEMS = [i % NB for i in range(NB * repeat)]

    with tile.TileContext(nc) as tc, ExitStack() as ctx:
        const = ctx.enter_context(tc.tile_pool(name="const", bufs=1))
        ld = ctx.enter_context(tc.tile_pool(name="ld", bufs=2))
        c16 = ctx.enter_context(tc.tile_pool(name="c16", bufs=2))
        epool = ctx.enter_context(tc.tile_pool(name="epool", bufs=2))
        pt = ctx.enter_context(tc.tile_pool(name="pt", bufs=4))
        outp = ctx.enter_context(tc.tile_pool(name="outp", bufs=6))
        small = ctx.enter_context(tc.tile_pool(name="small", bufs=2))
        # PSUM: mm 2x[P,2,F](4 banks) + pv 3x[P,F](3) + rsc 1 = 8
        ps_mm = ctx.enter_context(tc.tile_pool(name="ps_mm", bufs=2, space="PSUM"))
        ps_pv = ctx.enter_context(tc.tile_pool(name="ps_pv", bufs=3, space="PSUM"))
        ps_rsc = ctx.enter_context(tc.tile_pool(name="ps_rsc", bufs=1, space="PSUM"))

        def cp(e, dst, src):
            if e is nc.scalar:
                e.copy(dst, src)
            else:
                e.tensor_copy(dst, src)

        # ---- constants ----
        ident16 = const.tile([P, P], f16, tag="ident16")
        make_identity(nc, ident16)
        ones21 = const.tile([P, 2, 1], f8, tag="ones21")
        nc.vector.memset(ones21[:], 1.0)
        bias_col = {}
        bv_bc = None
        if apply_qkv_bias:
            cpack = const.tile([P, 16], f32, tag="cpack")
            for i, n in enumerate(("bq", "bk")):
                dst = cpack[:, 4 * i: 4 * (i + 1)]
                nc.sync.dma_start(dst, bias_d[n].rearrange("(o p) -> p o", p=P))
                bias_col[n] = dst
            bv_bc = const.tile([P, L], f32, tag="bv_bc")
            nc.sync.dma_start(bv_bc[:], bcast_p(bias_d["bv"]))
        if apply_gamma_beta:
            gb_pack = const.tile([P, 2, L], f32, tag="gb")
            nc.sync.dma_start(gb_pack[:, 0, :], bcast_p(gamma_d))
            nc.sync.dma_start(gb_pack[:, 1, :], bcast_p(beta_d))

        WT = {n: const.tile([P, NLC, L], f8, tag=f"WT_{n}", name=f"WT_{n}")
              for n in ("Wq", "Wk", "Wv")}

        # ---------- emission helpers (lists of closures; each closure
        # emits one "block" of instructions) ----------

        w_eng = [nc.vector, nc.scalar]

        def load_weight_blocks(n, wi):
            # DMA W -> cast f16 -> 16 transposes + 4 copies into WT[n]
            def emit():
                wld = ld.tile([P, NLC, F], f32, tag="ld", name=f"wld_{n}")
                nc.sync.dma_start(wld[:],
                                  w_d[n].rearrange("(o p) l -> p o l", p=P))
                w16 = c16.tile([P, NLC, F], f16, tag="w16", bufs=1,
                               name=f"w16_{n}")
                nc.vector.tensor_copy(w16[:], wld[:])
                for li in range(NLC):
                    pst = ps_mm.tile([P, F], f16, tag="mm", name="wtp")
                    for mo in range(NLC):
                        nc.tensor.transpose(pst[:, ts(mo, P)],
                                            w16[:, mo, ts(li, P)], ident16[:])
                    cp(w_eng[(wi + li) % 2], WT[n][:, li, :], pst[:])
            return [emit]

        # per-element state holders
        class E:
            pass

        def make_elem(bi, rep):
            e = E()
            e.bi = bi
            e.rep = rep
            e.a8 = epool.tile([P, NLC, C], f8, tag="a8", name=f"a8_{rep}")
            e.b8 = epool.tile([P, NLC, C], f8, tag="b8", name=f"b8_{rep}")
            e.qT = epool.tile([P, NLC, C], f8, tag="qT", name=f"qT_{rep}")
            e.kT = epool.tile([P, NLC, C], f8, tag="kT", name=f"kT_{rep}")
            e.v8 = epool.tile([P, NDB, L], f8, tag="v8", name=f"v8_{rep}")
            e.apT = epool.tile([P, NDB, L], f16, tag="apT", name=f"apT_{rep}")
            e.a16 = {}
            return e

        # gpsimd casts are ~7us for [P, C]: too slow for deadline paths.
        a_cast8 = [nc.vector, nc.scalar, nc.vector, nc.scalar]
        b_cast8 = [nc.scalar, nc.vector, nc.scalar, nc.vector]
        apt_cp = [nc.vector, nc.scalar, nc.vector, nc.scalar]

        def load_a_chunk(e, li):
            # DMA a chunk; cast f16 + f8; keep a16 for transpose blocks
            def emit():
                ach = ld.tile([P, C], f32, tag="ld", name=f"a_{e.rep}_{li}")
                nc.sync.dma_start(ach[:], a_d[e.bi, ds(li * P, P), :])
                a16 = c16.tile([P, C], f16, tag="a16", bufs=4,
                               name=f"a16_{e.rep}_{li}")
                nc.vector.tensor_copy(a16[:], ach[:])
                cp(a_cast8[li], e.a8[:, li, :], ach[:])
                e.a16[li] = a16
            return [emit]

        def apt_blocks(e, li):
            # 4 transposes + 1 copy per block, 4 blocks per l-chunk li
            def mk(g):
                def emit():
                    a16 = e.a16[li]
                    pst = ps_mm.tile([P, F], f16, tag="mm", name="atp")
                    for j in range(4):
                        db = 4 * g + j
                        nc.tensor.transpose(pst[:, ts(j, P)],
                                            a16[:, ts(db, P)], ident16[:])
                    dst = e.apT[:, 4 * g:4 * g + 4, ts(li, P)]
                    cp(apt_cp[(li + g) % 2],
                       dst, pst[:].rearrange("p (b f) -> p b f", f=P))
                return emit
            return [mk(g) for g in range(NLC)]

        def load_b_chunk(e, li):
            def emit():
                bch = ld.tile([P, C], f32, tag="ld", name=f"b_{e.rep}_{li}")
                nc.sync.dma_start(bch[:], b_d[e.bi, ds(li * P, P), :])
                cp(b_cast8[li], e.b8[:, li, :], bch[:])
            return [emit]

        qk_cp = [nc.vector, nc.scalar]

        def qk_proj_blocks(e, wname, bname, src_name, dst_name):
            # per mi: 4 chains (ci) of 2 DR mms into [P,F] psums + 4 copies
            def mk(mi):
                def emit():
                    src = getattr(e, src_name)
                    dst = getattr(e, dst_name)
                    pss = [ps_mm.tile([P, F], f32, tag="mm", name=f"qk{i}")
                           for i in range(4)]
                    for lp in range(NLP):
                        for ci in range(NCCH):
                            nc.tensor.matmul(
                                pss[ci][:],
                                lhsT=WT[wname][:, 2 * lp:2 * lp + 2, ts(mi, P)],
                                rhs=src[:, 2 * lp:2 * lp + 2, ts(ci, F)],
                                start=(lp == 0), stop=(lp == NLP - 1),
                                perf_mode=DR)
                    for ci in range(NCCH):
                        dslice = dst[:, mi, ts(ci, F)]
                        if apply_qkv_bias:
                            nc.scalar.activation(
                                dslice, pss[ci][:], AF.Identity,
                                bias=bias_col[bname][:, mi:mi + 1])
                        else:
                            cp(qk_cp[ci % 2], dslice, pss[ci][:])
                return emit
            return [mk(mi) for mi in range(NLC)]

        def v_proj_blocks(e):
            # per dp: 2 chains (s) of 2 DR mms + 2 copies
            def mk(dp):
                def emit():
                    pss = [ps_mm.tile([P, F], f32, tag="mm", name=f"v{i}")
                           for i in range(2)]
                    for s in range(2):
                        di = 2 * dp + s
                        for lp in range(NLP):
                            nc.tensor.matmul(
                                pss[s][:],
                                lhsT=e.b8[:, 2 * lp:2 * lp + 2, ts(di, P)],
                                rhs=WT["Wv"][:, 2 * lp:2 * lp + 2, :],
                                start=(lp == 0), stop=(lp == NLP - 1),
                                perf_mode=DR)
                    for s in range(2):
                        cp(qk_cp[(dp + s) % 2], e.v8[:, 2 * dp + s, :],
                           pss[s][:])
                        if apply_qkv_bias:
                            nc.vector.tensor_add(e.v8[:, 2 * dp + s, :],
                                                 e.v8[:, 2 * dp + s, :],
                                                 bv_bc[:, :])
                return emit
            return [mk(dp) for dp in range(NDP)]

        # ---------- attention stage machinery ----------

        class Stage:
            # one (elem, p) pair: scores+exp+rowsum for c-chunk pair p;
            # PV+LN ride the NEXT stage's rounds.
            def __init__(s, e, p):
                s.e, s.p = e, p
                s.PT = [pt.tile([P, NDB, F], f8, tag="pt",
                                name=f"pt_{e.rep}_{p}_{j}") for j in range(2)]
                s.rsc = None
                s.stats = small.tile([P, 2, NCCH, 2], f32, tag="stats",
                                     name=f"stats_{e.rep}_{p}")
                s.outs = {}

        def scores_round(s, dp):
            # 8 DR mms -> 2 [P,2,F] psums; 2 exps (Act)
            def emit():
                e = s.e
                pss = [ps_mm.tile([P, 2, F], f32, tag="mm", name=f"sc{j}")
                       for j in range(2)]
                for j in range(2):
                    for sb in range(2):
                        di = 2 * dp + sb
                        for mp in range(NLP):
                            nc.tensor.matmul(
                                pss[j][:, sb, :],
                                lhsT=e.kT[:, 2 * mp:2 * mp + 2, ts(di, P)],
                                rhs=e.qT[:, 2 * mp:2 * mp + 2,
                                         ts(2 * s.p + j, F)],
                                start=(mp == 0), stop=(mp == NLP - 1),
                                perf_mode=DR)
                for j in range(2):
                    nc.scalar.activation(s.PT[j][:, 2 * dp:2 * dp + 2, :],
                                         pss[j][:], AF.Exp, scale=INV_SQRT_L)
            return emit

        def pv_chain(s, idx):
            # idx in 0..7 -> (j, cb). 8 DR mms + stt + bn_stats (DVE)
            j, cb = idx // NCCH, idx % NCCH

            def emit():
                e = s.e
                po = ps_pv.tile([P, L], f32, tag="pv", name="po")
                if s.rsc is None:
                    s.rsc = ps_rsc.tile([P, 2, NCCH], f32, tag="rsc",
                                        name="rsc")
                for dp in range(NDP):
                    lw = s.PT[j][:, 2 * dp:2 * dp + 2, ts(cb, P)]
                    nc.tensor.matmul(
                        po[:], lhsT=lw,
                        rhs=e.v8[:, 2 * dp:2 * dp + 2, :],
                        start=(dp == 0), stop=(dp == NDP - 1),
                        perf_mode=DR)
                    nc.tensor.matmul(
                        s.rsc[:, j, cb:cb + 1], lhsT=lw, rhs=ones21[:],
                        start=(dp == 0), stop=(dp == NDP - 1),
                        perf_mode=DR, skip_group_check=True)
                gb = (2 * s.p + j) * NCCH + cb
                out_sb = outp.tile([P, L], f32, tag="out", name="out_sb")
                nc.vector.scalar_tensor_tensor(
                    out_sb[:], e.apT[:, gb, :],
                    s.rsc[:, j, cb:cb + 1], po[:], ALU.mult, ALU.add)
                st6 = small.tile([P, 6], f32, tag="st6", name="st6")
                nc.vector.bn_stats(st6[:], out_sb[:])
                nc.vector.bn_aggr(s.stats[:, j, cb, :], st6[:])
                s.outs[idx] = out_sb
            return emit

        def ln_finish(s, j):
            # batched Newton rsqrt for the 4 cb's of j; final scale + DMA
            def emit():
                e = s.e
                var_ap = s.stats[:, j, :, 1]
                y = small.tile([P, 2, NCCH], f32, tag="nwt", name="nwt")
                nc.vector.tensor_scalar(y[:, 0, :], var_ap,
                                        -0.5 * RSTD_SEED ** 3,
                                        1.5 * RSTD_SEED,
                                        ALU.mult, ALU.add)
                for _ in range(3):
                    t = y[:, 1, :]
                    nc.vector.tensor_mul(t, y[:, 0, :], y[:, 0, :])
                    nc.vector.tensor_mul(t, t, var_ap)
                    nc.vector.tensor_scalar(t, t, -0.5, 1.5,
                                            ALU.mult, ALU.add)
                    nc.vector.tensor_mul(y[:, 0, :], y[:, 0, :], t)
                for cb in range(NCCH):
                    gb = (2 * s.p + j) * NCCH + cb
                    out_sb = s.outs[j * NCCH + cb]
                    nc.vector.tensor_scalar(out_sb[:], out_sb[:],
                                            s.stats[:, j, cb, 0:1],
                                            y[:, 0, cb:cb + 1],
                                            ALU.subtract, ALU.mult)
                    if apply_gamma_beta:
                        nc.vector.tensor_mul(out_sb[:], out_sb[:],
                                             gb_pack[:, 0, :])
                        nc.vector.tensor_add(out_sb[:], out_sb[:],
                                             gb_pack[:, 1, :])
                    nc.sync.dma_start(out_d[e.bi, ds(gb * P, P), :],
                                      out_sb[:])
            return emit

        def stage_rounds(s, prev, fillers):
            # 8 rounds: scores(dp) + rs(dp-1) + prev's pv chain + fillers
            blocks = []
            for dp in range(NDP):
                blocks.append(scores_round(s, dp))
                if prev is not None:
                    blocks.append(pv_chain(prev, dp))
                    if dp == NCCH - 1:
                        blocks.append(ln_finish(prev, 0))
                    if dp == 2 * NCCH - 1:
                        blocks.append(ln_finish(prev, 1))
                if fillers:
                    blocks.append(fillers.pop(0))
                    if len(fillers) > 2 * (NDP - 1 - dp):
                        blocks.append(fillers.pop(0))
            if fillers:
                blocks.append(fillers.pop(0))
            return blocks

        def drain_stage(s):
            # pv+ln for the final stage (no next stage to ride in)
            blocks = []
            for idx in range(2 * NCCH):
                blocks.append(pv_chain(s, idx))
                if idx == NCCH - 1:
                    blocks.append(ln_finish(s, 0))
            blocks.append(ln_finish(s, 1))
            return blocks

        # ---------- schedule ----------
        elems = [make_elem(bi, rep) for rep, bi in enumerate(ELEMS)]

        def load_a_blocks(e):
            blocks = []
            for li in range(NLC):
                blocks += load_a_chunk(e, li)
            return blocks

        def load_b_blocks(e):
            blocks = []
            for li in range(NLC):
                blocks += load_b_chunk(e, li)
            return blocks

        def elem_apt_blocks(e):
            blocks = []
            for li in range(NLC):
                blocks += apt_blocks(e, li)
            return blocks

        def elem_proj_blocks(e):
            # q first (a-gated), then k/v interleaved (b-gated)
            qb = qk_proj_blocks(e, "Wq", "bq", "a8", "qT")
            kb = qk_proj_blocks(e, "Wk", "bk", "b8", "kT")
            vb = v_proj_blocks(e)
            blocks = list(qb)
            for i in range(NLC):
                blocks.append(kb[i])
                blocks.append(vb[2 * i])
                blocks.append(vb[2 * i + 1])
            return blocks

        # head: weights + elem0 loads + apT0 + proj0.  DMA order
        # interleaves Wk/Wv into the a-stream so q can start early.
        e0 = elems[0]
        la = load_a_blocks(e0)
        head = []
        head += load_weight_blocks("Wq", 0)
        head += [la[0], la[1]]
        head += load_weight_blocks("Wk", 1)
        head += [la[2], la[3]]
        head += load_weight_blocks("Wv", 0)
        head += load_b_blocks(e0)
        head += qk_proj_blocks(e0, "Wq", "bq", "a8", "qT")
        head += elem_apt_blocks(e0)
        kb0 = qk_proj_blocks(e0, "Wk", "bk", "b8", "kT")
        vb0 = v_proj_blocks(e0)
        for i in range(NLC):
            head.append(kb0[i])
            head.append(vb0[2 * i])
            head.append(vb0[2 * i + 1])
        for blk in head:
            blk()

        # stages pipeline: next elem's a-side prep rides p=0, b-side + proj
        # ride p=1.
        prev_stage = None
        for ei, e in enumerate(elems):
            en = elems[ei + 1] if ei + 1 < len(elems) else None
            for p in range(2):
                s = Stage(e, p)
                if en is None:
                    fillers = []
                elif p == 0:
                    fillers = load_a_blocks(en) + elem_apt_blocks(en)
                else:
                    fillers = load_b_blocks(en) + elem_proj_blocks(en)
                for blk in stage_rounds(s, prev_stage, fillers):
                    blk()
                for blk in fillers:
                    blk()
                prev_stage = s
        for blk in drain_stage(prev_stage):
            blk()

    nc.compile()
    return nc


def _get_nc(apply_qkv_bias, apply_gamma_beta, repeat=1):
    key = (apply_qkv_bias, apply_gamma_beta, repeat)
    if key not in _CACHE:
        _CACHE[key] = _build(*key)
    return _CACHE[key]


def _run(inputs, trace=False):
    from concourse import bass_utils

    a = np.ascontiguousarray(np.asarray(inputs["a"], dtype=np.float32))
    b = np.ascontiguousarray(np.asarray(inputs["b"], dtype=np.float32))
    get = lambda n: np.ascontiguousarray(np.asarray(inputs[n], dtype=np.float32))
    Wq, Wk, Wv = get("Wq"), get("Wk"), get("Wv")
    bq, bk, bv = get("bq"), get("bk"), get("bv")
    gamma, beta = get("gamma"), get("beta")

    apply_qkv_bias = bool(np.any(bq) or np.any(bk) or np.any(bv))
    apply_gamma_beta = bool(np.any(gamma != 1.0) or np.any(beta))
    nc = _get_nc(apply_qkv_bias, apply_gamma_beta)

    in_maps = []
    for c in range(NCORE):
        sl = slice(c * NB, (c + 1) * NB)
        in_maps.append({
            "a": np.ascontiguousarray(a[sl]), "b": np.ascontiguousarray(b[sl]),
            "Wq": Wq, "Wk": Wk, "Wv": Wv,
            "bq": bq, "bk": bk, "bv": bv,
            "gamma": gamma, "beta": beta,
        })
    res = bass_utils.run_bass_kernel_spmd(nc, in_maps,
                                          core_ids=list(range(NCORE)),
                                          trace=trace)
    out = np.concatenate(
        [res.results[c]["out"].reshape(NB, L, C) for c in range(NCORE)], axis=0)
    return out, res


def kernel(**inputs):
    out, _ = _run(inputs, trace=False)
    return out


# revision 11
# speedup vs baseline: 1.0105x; 1.0105x over previous
"""CrossDomainAttention TRN2 kernel: 8-core data-parallel over batch.

Reference computation (per batch element, a/b are (L, C) slices):
  ap = a.T (C, L);  q = ap@Wq.T+bq; k,v from b.T
  attn = softmax(q @ k.T / sqrt(L)) (C, C)
  out = LN(attn @ v + ap) over L, returned as the raw (C*L) buffer viewed (L, C)

v2: fp8 (e4m3) DoubleRow with a 2-deep software pipeline keeping the PE
dense (HAM stays warm).  Each attention "round" interleaves on the PE:
scores(stage s, dp) + PE row-sum(dp-1) + one PV chain of stage s-1, so
the Act engine's exp drain (~2.2us/round) always fits inside the PE
round (~3.8us).  The next element's DMA/casts/transposes/projections are
emitted as fillers inside earlier stages (a-side into p=0, b-side+proj
into p=1).  LN uses the scale-invariant form (out_pre = rowsum*apT + PV)
with a batched Newton rsqrt.  Residual apT is fp16, transposed from an
fp16 copy of a at 1 cyc/row.
"""

import numpy as np

B, L, C = 16, 512, 2048
NCORE = 8
NB = B // NCORE          # batch elements per core
P = 128
F = 512                  # matmul free-dim tile
NLC = L // P             # 4  l/m chunks
NDB = C // P             # 16 d-blocks / c-blocks
NCCH = C // F            # 4  c chunks
NDP = NDB // 2           # 8  d-pairs (DoubleRow)
NLP = NLC // 2           # 2  l/m pairs (DoubleRow)
LN_EPS = 1e-5
RSTD_SEED = 4.77e-4   # ~1/sqrt(mean var') for the scale-invariant LN form
INV_SQRT_L = 1.0 / float(np.sqrt(L))

_CACHE = {}


def _build(apply_qkv_bias: bool, apply_gamma_beta: bool, repeat: int = 1):
    import concourse.bass as bass
    import concourse.tile as tile
    from concourse import bacc, mybir
    from concourse.bass import ts, ds
    from concourse.masks import make_identity
    from contextlib import ExitStack

    f32 = mybir.dt.float32
    f16 = mybir.dt.float16
    f8 = mybir.dt.float8e4
    AF = mybir.ActivationFunctionType
    ALU = mybir.AluOpType
    DR = mybir.MatmulPerfMode.DoubleRow

    nc = bacc.Bacc("TRN2", target_bir_lowering=False, debug=False,
                   enable_asserts=False)

    a_d = nc.dram_tensor("a", (NB, L, C), f32, kind="ExternalInput").ap()
    b_d = nc.dram_tensor("b", (NB, L, C), f32, kind="ExternalInput").ap()
    w_d = {n: nc.dram_tensor(n, (L, L), f32, kind="ExternalInput").ap()
           for n in ("Wq", "Wk", "Wv")}
    bias_d = {n: nc.dram_tensor(n, (L,), f32, kind="ExternalInput").ap()
              for n in ("bq", "bk", "bv")}
    gamma_d = nc.dram_tensor("gamma", (L,), f32, kind="ExternalInput").ap()
    beta_d = nc.dram_tensor("beta", (L,), f32, kind="ExternalInput").ap()
    out_d = nc.dram_tensor("out", (NB, C, L), f32, kind="ExternalOutput").ap()

    def bcast_p(ap1d):
        # broadcast a 1-D DRAM AP across all 128 partitions (DMA source)
        return bass.AP(tensor=ap1d.tensor, offset=ap1d.offset,
                       ap=[[0, P]] + [list(d) for d in ap1d.ap])

    ELEMS = [i % NB for i in range(NB * repeat)]

    with tile.TileContext(nc) as tc, ExitStack() as ctx:
        const = ctx.enter_context(tc.tile_pool(name="const", bufs=1))
        ld = ctx.enter_context(tc.tile_pool(name="ld", bufs=2))
        c16 = ctx.enter_context(tc.tile_pool(name="c16", bufs=2))
        epool = ctx.enter_context(tc.tile_pool(name="epool", bufs=2))
        pt = ctx.enter_context(tc.tile_pool(name="pt", bufs=4))
        outp = ctx.enter_context(tc.tile_pool(name="outp", bufs=6))
        small = ctx.enter_context(tc.tile_pool(name="small", bufs=2))
        # PSUM: mm 2x[P,2,F](4 banks) + pv 4x[P,F](4) = 8.  Row-sum psums
        # borrow "mm" slots briefly at stage end.
        ps_mm = ctx.enter_context(tc.tile_pool(name="ps_mm", bufs=2, space="PSUM"))
        ps_pv = ctx.enter_context(tc.tile_pool(name="ps_pv", bufs=4, space="PSUM"))

        def cp(e, dst, src):
            if e is nc.scalar:
                e.copy(dst, src)
            else:
                e.tensor_copy(dst, src)

        # ---- constants ----
        ident16 = const.tile([P, P], f16, tag="ident16")
        make_identity(nc, ident16)
        ones2 = const.tile([P, 2, 16], f8, tag="ones2")
        nc.vector.memset(ones2[:], 1.0)
        ident1 = const.tile([1, 1], f32, tag="ident1")
        nc.vector.memset(ident1[:], 1.0)
        bias_col = {}
        bv_bc = None
        if apply_qkv_bias:
            cpack = const.tile([P, 16], f32, tag="cpack")
            for i, n in enumerate(("bq", "bk")):
                dst = cpack[:, 4 * i: 4 * (i + 1)]
                nc.sync.dma_start(dst, bias_d[n].rearrange("(o p) -> p o", p=P))
                bias_col[n] = dst
            bv_bc = const.tile([P, L], f32, tag="bv_bc")
            nc.sync.dma_start(bv_bc[:], bcast_p(bias_d["bv"]))
        if apply_gamma_beta:
            gb_pack = const.tile([P, 2, L], f32, tag="gb")
            nc.sync.dma_start(gb_pack[:, 0, :], bcast_p(gamma_d))
            nc.sync.dma_start(gb_pack[:, 1, :], bcast_p(beta_d))

        WT = {n: const.tile([P, NLC, L], f8, tag=f"WT_{n}", name=f"WT_{n}")
              for n in ("Wq", "Wk", "Wv")}

        # ---------- emission helpers (lists of closures; each closure
        # emits one "block" of instructions) ----------

        w_eng = [nc.vector, nc.scalar]

        def load_weight_blocks(n, wi):
            # DMA W -> cast f16 -> 16 transposes + 4 copies into WT[n]
            def emit():
                wld = ld.tile([P, NLC, F], f32, tag="ld", name=f"wld_{n}")
                nc.sync.dma_start(wld[:],
                                  w_d[n].rearrange("(o p) l -> p o l", p=P))
                w16 = c16.tile([P, NLC, F], f16, tag="w16", bufs=1,
                               name=f"w16_{n}")
                nc.vector.tensor_copy(w16[:], wld[:])
                for li in range(NLC):
                    pst = ps_mm.tile([P, F], f16, tag="mm", name="wtp")
                    for mo in range(NLC):
                        nc.tensor.transpose(pst[:, ts(mo, P)],
                                            w16[:, mo, ts(li, P)], ident16[:])
                    cp(w_eng[(wi + li) % 2], WT[n][:, li, :], pst[:])
            return [emit]

        # per-element state holders
        class E:
            pass

        def make_elem(bi, rep):
            e = E()
            e.bi = bi
            e.rep = rep
            e.a8 = epool.tile([P, NLC, C], f8, tag="a8", name=f"a8_{rep}")
            e.b8 = epool.tile([P, NLC, C], f8, tag="b8", name=f"b8_{rep}")
            e.qT = epool.tile([P, NLC, C], f8, tag="qT", name=f"qT_{rep}")
            e.kT = epool.tile([P, NLC, C], f8, tag="kT", name=f"kT_{rep}")
            e.v8 = epool.tile([P, NDB, L], f8, tag="v8", name=f"v8_{rep}")
            e.apT = epool.tile([P, NDB, L], f16, tag="apT", name=f"apT_{rep}")
            e.a16 = {}
            return e

        # gpsimd casts are ~7us for [P, C]: too slow for deadline paths.
        a_cast8 = [nc.vector, nc.scalar, nc.vector, nc.scalar]
        b_cast8 = [nc.scalar, nc.vector, nc.scalar, nc.vector]
        apt_cp = [nc.vector, nc.scalar, nc.vector, nc.scalar]

        def load_a_chunk(e, li):
            # DMA a chunk; cast f16 + f8; keep a16 for transpose blocks
            def emit():
                ach = ld.tile([P, C], f32, tag="ld", name=f"a_{e.rep}_{li}")
                nc.sync.dma_start(ach[:], a_d[e.bi, ds(li * P, P), :])
                a16 = c16.tile([P, C], f16, tag="a16", bufs=4,
                               name=f"a16_{e.rep}_{li}")
                nc.vector.tensor_copy(a16[:], ach[:])
                cp(a_cast8[li], e.a8[:, li, :], ach[:])
                e.a16[li] = a16
            return [emit]

        def apt_blocks(e, li):
            # 4 transposes + 1 copy per block, 4 blocks per l-chunk li
            def mk(g):
                def emit():
                    a16 = e.a16[li]
                    pst = ps_mm.tile([P, F], f16, tag="mm", name="atp")
                    for j in range(4):
                        db = 4 * g + j
                        nc.tensor.transpose(pst[:, ts(j, P)],
                                            a16[:, ts(db, P)], ident16[:])
                    dst = e.apT[:, 4 * g:4 * g + 4, ts(li, P)]
                    cp(apt_cp[(li + g) % 2],
                       dst, pst[:].rearrange("p (b f) -> p b f", f=P))
                return emit
            return [mk(g) for g in range(NLC)]

        def load_b_chunk(e, li):
            def emit():
                bch = ld.tile([P, C], f32, tag="ld", name=f"b_{e.rep}_{li}")
                nc.sync.dma_start(bch[:], b_d[e.bi, ds(li * P, P), :])
                cp(b_cast8[li], e.b8[:, li, :], bch[:])
            return [emit]

        qk_cp = [nc.vector, nc.scalar]

        def qk_proj_blocks(e, wname, bname, src_name, dst_name):
            # per mi: 4 chains (ci) of 2 DR mms into [P,F] psums + 4 copies
            def mk(mi):
                def emit():
                    src = getattr(e, src_name)
                    dst = getattr(e, dst_name)
                    pss = [ps_mm.tile([P, F], f32, tag="mm", name=f"qk{i}")
                           for i in range(4)]
                    for lp in range(NLP):
                        for ci in range(NCCH):
                            nc.tensor.matmul(
                                pss[ci][:],
                                lhsT=WT[wname][:, 2 * lp:2 * lp + 2, ts(mi, P)],
                                rhs=src[:, 2 * lp:2 * lp + 2, ts(ci, F)],
                                start=(lp == 0), stop=(lp == NLP - 1),
                                perf_mode=DR)
                    for ci in range(NCCH):
                        dslice = dst[:, mi, ts(ci, F)]
                        if apply_qkv_bias:
                            nc.scalar.activation(
                                dslice, pss[ci][:], AF.Identity,
                                bias=bias_col[bname][:, mi:mi + 1])
                        else:
                            cp(qk_cp[ci % 2], dslice, pss[ci][:])
                return emit
            return [mk(mi) for mi in range(NLC)]

        def v_proj_blocks(e):
            # per dp: 2 chains (s) of 2 DR mms + 2 copies
            def mk(dp):
                def emit():
                    pss = [ps_mm.tile([P, F], f32, tag="mm", name=f"v{i}")
                           for i in range(2)]
                    for s in range(2):
                        di = 2 * dp + s
                        for lp in range(NLP):
                            nc.tensor.matmul(
                                pss[s][:],
                                lhsT=e.b8[:, 2 * lp:2 * lp + 2, ts(di, P)],
                                rhs=WT["Wv"][:, 2 * lp:2 * lp + 2, :],
                                start=(lp == 0), stop=(lp == NLP - 1),
                                perf_mode=DR)
                    for s in range(2):
                        cp(qk_cp[(dp + s) % 2], e.v8[:, 2 * dp + s, :],
                           pss[s][:])
                        if apply_qkv_bias:
                            nc.vector.tensor_add(e.v8[:, 2 * dp + s, :],
                                                 e.v8[:, 2 * dp + s, :],
                                                 bv_bc[:, :])
                return emit
            return [mk(dp) for dp in range(NDP)]

        # ---------- attention stage machinery ----------

        class Stage:
            # one (elem, p) pair: scores+exp+rowsum for c-chunk pair p;
            # PV+LN ride the NEXT stage's rounds.
            def __init__(s, e, p):
                s.e, s.p = e, p
                s.PT = [pt.tile([P, NDB, F], f8, tag="pt",
                                name=f"pt_{e.rep}_{p}_{j}") for j in range(2)]
                s.rs_cols = None
                s.stats = small.tile([P, 2, NCCH, 2], f32, tag="stats",
                                     name=f"stats_{e.rep}_{p}")
                s.outs = {}

        def scores_round(s, dp):
            # 8 DR mms -> 2 [P,2,F] psums; 2 exps (Act)
            def emit():
                e = s.e
                pss = [ps_mm.tile([P, 2, F], f32, tag="mm", name=f"sc{j}")
                       for j in range(2)]
                for j in range(2):
                    for sb in range(2):
                        di = 2 * dp + sb
                        for mp in range(NLP):
                            nc.tensor.matmul(
                                pss[j][:, sb, :],
                                lhsT=e.kT[:, 2 * mp:2 * mp + 2, ts(di, P)],
                                rhs=e.qT[:, 2 * mp:2 * mp + 2,
                                         ts(2 * s.p + j, F)],
                                start=(mp == 0), stop=(mp == NLP - 1),
                                perf_mode=DR)
                for j in range(2):
                    nc.scalar.activation(s.PT[j][:, 2 * dp:2 * dp + 2, :],
                                         pss[j][:], AF.Exp, scale=INV_SQRT_L)
            return emit

        def rs_block(s):
            # 16 N=512 ones-lhsT mms -> psr rows; then psrow copy (Act),
            # 8 tiny transposes, and a copy of the columns to SBUF.
            def emit_mms():
                s.psr = ps_mm.tile([16, 2, F], f32, tag="mm", name="psr")
                for j in range(2):
                    for dp in range(NDP):
                        nc.tensor.matmul(s.psr[:, j, :],
                                         lhsT=ones2[:],
                                         rhs=s.PT[j][:, 2 * dp:2 * dp + 2, :],
                                         start=(dp == 0), stop=(dp == NDP - 1),
                                         perf_mode=DR,
                                         skip_group_check=True)

            def emit_post():
                psrow = small.tile([1, 2, F], f32, tag="rrow", bufs=1,
                                   name="psrow")
                for j in range(2):
                    nc.scalar.copy(psrow[:, j, :], s.psr[0:1, j, :])
                trs_ps = ps_mm.tile([P, 2, NCCH], f32, tag="mm", name="trs")
                for j in range(2):
                    for cb in range(NCCH):
                        nc.tensor.transpose(trs_ps[:, j, cb:cb + 1],
                                            psrow[0:1, j, ts(cb, P)],
                                            ident1[:])
                s.rs_cols = small.tile([P, 2, NCCH], f32, tag="rcol",
                                       name="rs_cols")
                nc.vector.tensor_copy(s.rs_cols[:], trs_ps[:])
            return [emit_mms, emit_post]

        def pv_chain(s, idx):
            # idx in 0..7 -> (j, cb). 8 DR mms + stt + bn_stats (DVE)
            j, cb = idx // NCCH, idx % NCCH

            def emit():
                e = s.e
                po = ps_pv.tile([P, L], f32, tag="pv", name="po")
                for dp in range(NDP):
                    nc.tensor.matmul(
                        po[:],
                        lhsT=s.PT[j][:, 2 * dp:2 * dp + 2, ts(cb, P)],
                        rhs=e.v8[:, 2 * dp:2 * dp + 2, :],
                        start=(dp == 0), stop=(dp == NDP - 1),
                        perf_mode=DR)
                gb = (2 * s.p + j) * NCCH + cb
                out_sb = outp.tile([P, L], f32, tag="out", name="out_sb")
                nc.vector.scalar_tensor_tensor(
                    out_sb[:], e.apT[:, gb, :],
                    s.rs_cols[:, j, cb:cb + 1], po[:], ALU.mult, ALU.add)
                st6 = small.tile([P, 6], f32, tag="st6", name="st6")
                nc.vector.bn_stats(st6[:], out_sb[:])
                nc.vector.bn_aggr(s.stats[:, j, cb, :], st6[:])
                s.outs[idx] = out_sb
            return emit

        def ln_finish(s, j):
            # batched Newton rsqrt for the 4 cb's of j; final scale + DMA
            def emit():
                e = s.e
                var_ap = s.stats[:, j, :, 1]
                y = small.tile([P, 2, NCCH], f32, tag="nwt", name="nwt")
                nc.vector.tensor_scalar(y[:, 0, :], var_ap,
                                        -0.5 * RSTD_SEED ** 3,
                                        1.5 * RSTD_SEED,
                                        ALU.mult, ALU.add)
                for _ in range(3):
                    t = y[:, 1, :]
                    nc.vector.tensor_mul(t, y[:, 0, :], y[:, 0, :])
                    nc.vector.tensor_mul(t, t, var_ap)
                    nc.vector.tensor_scalar(t, t, -0.5, 1.5,
                                            ALU.mult, ALU.add)
                    nc.vector.tensor_mul(y[:, 0, :], y[:, 0, :], t)
                for cb in range(NCCH):
                    gb = (2 * s.p + j) * NCCH + cb
                    out_sb = s.outs[j * NCCH + cb]
                    nc.vector.tensor_scalar(out_sb[:], out_sb[:],
                                            s.stats[:, j, cb, 0:1],
                                            y[:, 0, cb:cb + 1],
                                            ALU.subtract, ALU.mult)
                    if apply_gamma_beta:
                        nc.vector.tensor_mul(out_sb[:], out_sb[:],
                                             gb_pack[:, 0, :])
                        nc.vector.tensor_add(out_sb[:], out_sb[:],
                                             gb_pack[:, 1, :])
                    nc.sync.dma_start(out_d[e.bi, ds(gb * P, P), :],
                                      out_sb[:])
            return emit

        def stage_rounds(s, prev, fillers):
            # 8 rounds: scores(dp) + rs(dp-1) + prev's pv chain + fillers
            blocks = []
            for dp in range(NDP):
                blocks.append(scores_round(s, dp))
                if prev is not None:
                    blocks.append(pv_chain(prev, dp))
                    if dp == NCCH - 1:
                        blocks.append(ln_finish(prev, 0))
                    if dp == 2 * NCCH - 1:
                        blocks.append(ln_finish(prev, 1))
                if fillers:
                    blocks.append(fillers.pop(0))
                    if len(fillers) > 2 * (NDP - 1 - dp):
                        blocks.append(fillers.pop(0))
            rsb = rs_block(s)
            blocks.append(rsb[0])
            if fillers:
                blocks.append(fillers.pop(0))
            blocks.append(rsb[1])
            while fillers:
                blocks.append(fillers.pop(0))
            return blocks

        def drain_stage(s):
            # pv+ln for the final stage (no next stage to ride in)
            blocks = []
            for idx in range(2 * NCCH):
                blocks.append(pv_chain(s, idx))
                if idx == NCCH - 1:
                    blocks.append(ln_finish(s, 0))
            blocks.append(ln_finish(s, 1))
            return blocks

        # ---------- schedule ----------
        elems = [make_elem(bi, rep) for rep, bi in enumerate(ELEMS)]

        def load_a_blocks(e):
            blocks = []
            for li in range(NLC):
                blocks += load_a_chunk(e, li)
            return blocks

        def load_b_blocks(e):
            blocks = []
            for li in range(NLC):
                blocks += load_b_chunk(e, li)
            return blocks

        def elem_apt_blocks(e):
            blocks = []
            for li in range(NLC):
                blocks += apt_blocks(e, li)
            return blocks

        def elem_proj_blocks(e):
            # q first (a-gated), then k/v interleaved (b-gated)
            qb = qk_proj_blocks(e, "Wq", "bq", "a8", "qT")
            kb = qk_proj_blocks(e, "Wk", "bk", "b8", "kT")
            vb = v_proj_blocks(e)
            blocks = list(qb)
            for i in range(NLC):
                blocks.append(kb[i])
                blocks.append(vb[2 * i])
                blocks.append(vb[2 * i + 1])
            return blocks

        # head: weights + elem0 loads + apT0 + proj0.  DMA order
        # interleaves Wk/Wv into the a-stream so q can start early.
        e0 = elems[0]
        la = load_a_blocks(e0)
        head = []
        head += load_weight_blocks("Wq", 0)
        head += [la[0], la[1]]
        head += load_weight_blocks("Wk", 1)
        head += [la[2], la[3]]
        head += load_weight_blocks("Wv", 0)
        head += load_b_blocks(e0)
        head += qk_proj_blocks(e0, "Wq", "bq", "a8", "qT")
        head += elem_apt_blocks(e0)
        kb0 = qk_proj_blocks(e0, "Wk", "bk", "b8", "kT")
        vb0 = v_proj_blocks(e0)
        for i in range(NLC):
            head.append(kb0[i])
            head.append(vb0[2 * i])
            head.append(vb0[2 * i + 1])
        for blk in head:
            blk()

        # stages pipeline: next elem's a-side prep rides p=0, b-side + proj
        # ride p=1.
        prev_stage = None
        for ei, e in enumerate(elems):
            en = elems[ei + 1] if ei + 1 < len(elems) else None
            for p in range(2):
                s = Stage(e, p)
                if en is None:
                    fillers = []
                elif p == 0:
                    fillers = load_a_blocks(en) + elem_apt_blocks(en)
                else:
                    fillers = load_b_blocks(en) + elem_proj_blocks(en)
                for blk in stage_rounds(s, prev_stage, fillers):
                    blk()
                for blk in fillers:
                    blk()
                prev_stage = s
        for blk in drain_stage(prev_stage):
            blk()

    nc.compile()
    return nc


def _get_nc(apply_qkv_bias, apply_gamma_beta, repeat=1):
    key = (apply_qkv_bias, apply_gamma_beta, repeat)
    if key not in _CACHE:
        _CACHE[key] = _build(*key)
    return _CACHE[key]


def _run(inputs, trace=False):
    from concourse import bass_utils

    a = np.ascontiguousarray(np.asarray(inputs["a"], dtype=np.float32))
    b = np.ascontiguousarray(np.asarray(inputs["b"], dtype=np.float32))
    get = lambda n: np.ascontiguousarray(np.asarray(inputs[n], dtype=np.float32))
    Wq, Wk, Wv = get("Wq"), get("Wk"), get("Wv")
    bq, bk, bv = get("bq"), get("bk"), get("bv")
    gamma, beta = get("gamma"), get("beta")

    apply_qkv_bias = bool(np.any(bq) or np.any(bk) or np.any(bv))
    apply_gamma_beta = bool(np.any(gamma != 1.0) or np.any(beta))
    nc = _get_nc(apply_qkv_bias, apply_gamma_beta)

    in_maps = []
    for c in range(NCORE):
        sl = slice(c * NB, (c + 1) * NB)
        in_maps.append({
            "a": np.ascontiguousarray(a[sl]), "b": np.ascontiguousarray(b[sl]),
            "Wq": Wq, "Wk": Wk, "Wv": Wv,
            "bq": bq, "bk": bk, "bv": bv,
            "gamma": gamma, "beta": beta,
        })
    res = bass_utils.run_bass_kernel_spmd(nc, in_maps,
                                          core_ids=list(range(NCORE)),
                                          trace=trace)
    out = np.concatenate(
        [res.results[c]["out"].reshape(NB, L, C) for c in range(NCORE)], axis=0)
    return out, res


def kernel(**inputs):
    out, _ = _run(inputs, trace=False)
    return out
